# revision 12
# baseline (speedup 1.0000x reference)
"""NeuMissBlock Trainium2 kernel.

h_{t+1} = obs * (h_t @ W.T) + h0, depth steps, obs = ~isnan(x),
h0 = obs*(x - mu). Data-parallel over 8 NeuronCores (4096 rows each).

Variant "q" (fp8 double-row): steps t < depth-2 run the PE in fp8-e4m3
DoubleRow mode (0.5 cycles/row, 256-deep contraction per instruction), with
the h0 skip-connection injected as an fp8 identity matmul (z8 = [I8 | 0]
stationary), so each psum group is pure-PE: 1 inject + 2 DR mains per
j-tile. The mask+evict (h8' = obs * psum -> fp8) alternates between a
direct DVE tensor_tensor (alpha) and an ACT fp8-evict + Pool mask-multiply
(beta) to balance the three elementwise engines. Step depth-2 injects a
bf16 h0T and evicts to bf16 h9; the final step runs batch-major in bf16:
psum = x0B - mu (identity + rank-1 injects) + h9.T @ W.T, masked by obsB,
DMA'd out in f32. End-to-end rel err ~7.7e-3 vs the 2e-2 harness gate.

Injected tensors only need to be NaN-free (the masked evict repairs
missing entries — an identity matmul would otherwise propagate 0*NaN=NaN
across whole psum columns), so h0T/x0B are built with a single 2x-rate
tensor_scalar min/max clamp (IEEE minNum maps NaN to the bound) instead
of memset+copy_predicated. h08 feeds step-0 mains directly so it IS fully
masked via copy_predicated into a DMA-pre-zeroed tile. x transposes run
in bf16 (1 PE cycle/row instead of 2), and both b-tiles of a pair share
each stationary operand back-to-back so Ldweights loads are deduped.
"""
import numpy as np

BATCH = 32768
F = 512
N_CORES = 8
ROWS = BATCH // N_CORES   # 4096
BT = 512                  # batch rows per b-tile
P = 128
NF = F // P               # 4 f-tiles / k-tiles

BEST_VARIANT = "q"

# beta (ACT+Pool) evict share: counter % DEN < NUM
BETA_NUM, BETA_DEN = 15, 32
CLAMP = 1024.0

_cache: dict = {}


def _build(rows: int, depth: int, variant: str = BEST_VARIANT):
    import concourse.tile as tile
    from concourse import bacc, mybir
    from concourse.masks import make_identity

    f32 = mybir.dt.float32
    bf16 = mybir.dt.bfloat16
    fp8 = mybir.dt.float8e4
    i16 = mybir.dt.int16
    DR = mybir.MatmulPerfMode.DoubleRow
    AF = mybir.ActivationFunctionType
    OP = mybir.AluOpType
    nbt = rows // BT
    assert rows % BT == 0

    nc = bacc.Bacc("TRN2", target_bir_lowering=False, debug=False,
                   num_devices=N_CORES)
    x_ap = nc.dram_tensor("x", [rows, F], f32, kind="ExternalInput").ap()
    mu_ap = nc.dram_tensor("mu", [F], f32, kind="ExternalInput").ap()
    w_ap = nc.dram_tensor("W", [F, F], f32, kind="ExternalInput").ap()
    out_ap = nc.dram_tensor("out", [rows, F], f32, kind="ExternalOutput").ap()

    if variant == "noop":
        with tile.TileContext(nc) as tc:
            with tc.tile_pool(name="sbuf", bufs=2) as pool:
                for bt in range(nbt):
                    for i in range(NF):
                        t = pool.tile([P, F], f32, tag="t", name="t")
                        nc.sync.dma_start(
                            t[:], x_ap[bt * BT + i * P: bt * BT + (i + 1) * P, :])
                        nc.sync.dma_start(
                            out_ap[bt * BT + i * P: bt * BT + (i + 1) * P, :], t[:])
        nc.compile()
        return nc

    assert variant == "q"

    with tile.TileContext(nc) as tc:
        with (
            tc.tile_pool(name="const", bufs=1) as cpool,
            tc.tile_pool(name="work", bufs=1) as wpool,
            tc.tile_pool(name="io", bufs=2) as iopool,
            tc.tile_pool(name="psum", bufs=2, space="PSUM") as pspool,
        ):
            # ---- constants ----
            ident = cpool.tile([P, P], f32, tag="ident", name="ident")
            make_identity(nc, ident[:])
            identB = cpool.tile([P, P], bf16, tag="identB", name="identB")
            nc.vector.tensor_scalar_mul(identB[:], ident[:], 1.0)
            z8 = cpool.tile([P, 2, P], fp8, tag="z8", name="z8")
            nc.gpsimd.memset(z8[:], 0.0)
            nc.vector.tensor_scalar_mul(z8[:, 0:1, :], ident[:], 1.0)

            mu_sb = cpool.tile([P, NF], f32, tag="mu", name="mu_sb")
            nc.sync.dma_start(mu_sb[:], mu_ap.rearrange("(t p) -> p t", p=P))
            negmu = cpool.tile([P, NF], f32, tag="negmu", name="negmu")
            nc.vector.tensor_scalar_mul(negmu[:], mu_sb[:], -1.0)

            mu_row = cpool.tile([1, F], f32, tag="mu_row", name="mu_row")
            nc.sync.dma_start(mu_row[:], mu_ap.rearrange("(o f) -> o f", o=1))
            nmrowB = cpool.tile([1, F], bf16, tag="nmrowB", name="nmrowB")
            nc.vector.tensor_scalar_mul(nmrowB[:], mu_row[:], -1.0)
            onesB = cpool.tile([1, P], bf16, tag="onesB", name="onesB")
            nc.gpsimd.memset(onesB[:], 1.0)

            zsrcB = None
            if depth == 1:
                zsrcB = cpool.tile([P, NF * BT], bf16, tag="zsrcB",
                                   name="zsrcB")
                for i in range(2):
                    nc.gpsimd.memset(zsrcB[:, i * 2 * BT:(i + 1) * 2 * BT],
                                     0.0)
            zsrc8 = cpool.tile([P, (NF + 1) * BT], fp8, tag="zsrc8",
                               name="zsrc8")
            nc.gpsimd.memset(zsrc8[:], 0.0)

            wB = cpool.tile([P, NF * F], f32, tag="wB", name="wB")
            wBb = cpool.tile([P, NF * F], bf16, tag="wBb", name="wBb")

            def w_loads():
                for kt in range(NF):
                    for ft in range(NF):
                        nc.sync.dma_start(
                            wB[:, ft * F + kt * P: ft * F + (kt + 1) * P],
                            w_ap[ft * P:(ft + 1) * P, kt * P:(kt + 1) * P])
                nc.vector.tensor_scalar_mul(wBb[:], wB[:], 1.0)

            wTb = cpool.tile([P, NF * F], bf16, tag="wTb", name="wTb")
            # w8dr[:, ktp, j, s, :] = fp8(W^T block kt=2*ktp+s, out-tile j)
            w8dr = cpool.tile([P, 2, NF, 2, P], fp8, tag="w8dr", name="w8dr")

            def w_transposes():
                for kt in range(NF):
                    psb = pspool.tile([P, BT], bf16, tag="psb", bufs=2,
                                      name="psb")
                    for ft in range(NF):
                        nc.tensor.transpose(
                            psb[:, ft * P:(ft + 1) * P],
                            wBb[:, ft * F + kt * P: ft * F + (kt + 1) * P],
                            identB[:])
                    nc.vector.tensor_scalar_mul(
                        wTb[:, kt * F:(kt + 1) * F], psb[:], 1.0)
                    for j in range(NF):
                        nc.scalar.activation(
                            w8dr[:, kt // 2, j, kt % 2, :],
                            psb[:, j * P:(j + 1) * P], AF.Identity)

            # ---- per-b-tile stages ----
            def load(bt, half):
                xB = iopool.tile([P, NF, F], f32, tag=f"xB{half}",
                                 name=f"xB{half}")
                for i in range(NF):
                    nc.sync.dma_start(
                        xB[:, i:i+1, :],
                        x_ap[bt * BT + i * P: bt * BT + (i + 1) * P, :])
                return xB

            def alloc_st(half, xB):
                xBb = wpool.tile([P, NF, F], bf16, tag=f"xBb{half}",
                                 bufs=2, name=f"xBb{half}")
                nc.vector.tensor_scalar_mul(xBb[:], xB[:], 1.0)
                obsT = wpool.tile([P, NF, BT], bf16, tag=f"obsT{half}",
                                  bufs=2, name=f"obsT{half}")
                cth = wpool.tile([P, NF, BT], bf16, tag=f"cth{half}",
                                 bufs=2, name=f"cth{half}")
                h08 = wpool.tile([P, NF + 1, BT], fp8, tag=f"h08{half}",
                                 bufs=2, name=f"h08{half}")
                h0T = wpool.tile([P, NF, BT], bf16, tag=f"h0T{half}",
                                 bufs=2, name=f"h0T{half}")
                h8A = wpool.tile([P, NF, BT], fp8, tag=f"h8A{half}",
                                 name=f"h8A{half}")
                h8B = wpool.tile([P, NF, BT], fp8, tag=f"h8B{half}",
                                 name=f"h8B{half}")
                h9 = wpool.tile([P, NF, BT], bf16, tag=f"h9{half}",
                                name=f"h9{half}")
                obsB = wpool.tile([P, NF, F], bf16, tag=f"obsB{half}",
                                  name=f"obsB{half}")
                x0B = wpool.tile([P, NF, F], bf16, tag=f"x0B{half}",
                                 name=f"x0B{half}")
                # pre-zero the h08 cpred destination (incl. the pad block)
                nc.sync.dma_start(h08[:], zsrc8[:])
                return dict(xBb=xBb, obsT=obsT, cth=cth, h08=h08, h8A=h8A,
                            h8B=h8B, h9=h9, h0T=h0T, obsB=obsB, x0B=x0B,
                            xB=xB, half=half)

            def h9_zero(st):
                nc.sync.dma_start(st["h9"][:], zsrcB[:])

            def setup_fm_chunk(st, j, startup=False):
                xBb, obsT, cth, h08 = (st["xBb"], st["obsT"], st["cth"],
                                       st["h08"])
                psb = pspool.tile([P, BT], bf16, tag="psb", bufs=2,
                                  name="psb")
                for i in range(NF):
                    nc.tensor.transpose(
                        psb[:, i * P:(i + 1) * P],
                        xBb[:, i, j * P:(j + 1) * P],
                        identB[:])
                nc.scalar.activation(cth[:, j, :], psb[:], AF.Identity,
                                     bias=negmu[:, j:j + 1])
                if startup:
                    # pair-0 burst: ACT is the serial bottleneck, use DVE
                    nc.vector.tensor_tensor(obsT[:, j, :], cth[:, j, :],
                                            cth[:, j, :], OP.is_equal)
                else:
                    nc.scalar.activation(obsT[:, j, :], psb[:], AF.Is_finite)
                nc.vector.copy_predicated(
                    h08[:, j:j+1, :], obsT[:, j:j+1, :].bitcast(i16),
                    cth[:, j:j+1, :])
                nc.vector.tensor_scalar(
                    st["h0T"][:, j:j+1, :], cth[:, j:j+1, :], CLAMP, -CLAMP,
                    OP.min, OP.max)

            def setup_bm_chunk(st, i):
                xB, obsB, x0B = st["xB"], st["obsB"], st["x0B"]
                nc.scalar.activation(obsB[:, i, :], xB[:, i, :], AF.Is_finite)
                nc.vector.tensor_scalar(
                    x0B[:, i:i+1, :], xB[:, i:i+1, :], CLAMP, -CLAMP,
                    OP.min, OP.max)

            ectr = [0]

            def evict_beta():
                ectr[0] += 1
                return (ectr[0] * BETA_NUM) % BETA_DEN < BETA_NUM

            def evict(st, ps2, dst, jj, last_fm=False):
                if evict_beta():
                    edt = bf16 if last_fm else fp8
                    e8 = wpool.tile([P, 2, BT], edt,
                                    tag=f"ebuf{1 if last_fm else 0}",
                                    bufs=3, name="ebuf")
                    nc.scalar.copy(e8[:], ps2[:])
                    nc.gpsimd.tensor_tensor(
                        dst[:, jj, :], e8[:], st["obsT"][:, jj, :], OP.mult)
                else:
                    nc.vector.tensor_tensor(
                        dst[:, jj, :], ps2[:], st["obsT"][:, jj, :], OP.mult)

            def round_fm_pair(sts, t):
                last_fm = (t == depth - 2)

                def src(st):
                    return st["h08"] if t == 0 else (
                        st["h8A"] if t % 2 == 1 else st["h8B"])

                def dst(st):
                    return st["h9"] if last_fm else (
                        st["h8A"] if t % 2 == 0 else st["h8B"])

                for jp in range(NF // 2):
                    ps2s = []
                    # injects issued contiguously so the stationary (z8 or
                    # identB) is loaded into the PE array once per jp
                    for h in (0, 1):
                        st = sts[h]
                        ps2 = pspool.tile([P, 2, BT], f32, tag="ps2", bufs=3,
                                          name="ps2")
                        ps2s.append(ps2)
                        for g in (0, 1):
                            j = 2 * jp + g
                            if last_fm:
                                nc.tensor.matmul(ps2[:, g:g+1, :], identB[:],
                                                 st["h0T"][:, j, :],
                                                 start=True, stop=False)
                            else:
                                nc.tensor.matmul(ps2[:, g:g+1, :], z8[:],
                                                 st["h08"][:, j:j+2, :],
                                                 start=True, stop=False,
                                                 perf_mode=DR)
                    # mains: both halves back-to-back per stationary block
                    for ktp in (0, 1):
                        for g in (0, 1):
                            j = 2 * jp + g
                            for h in (0, 1):
                                nc.tensor.matmul(
                                    ps2s[h][:, g:g+1, :], w8dr[:, ktp, j, :, :],
                                    src(sts[h])[:, 2*ktp:2*ktp+2, :],
                                    start=False, stop=(ktp == 1),
                                    perf_mode=DR, skip_group_check=True)
                    jj = slice(2 * jp, 2 * jp + 2)
                    for h in (0, 1):
                        evict(sts[h], ps2s[h], dst(sts[h]), jj, last_fm)

            def bm_group_pair(sts, bts, sp, last_drain=False):
                psBs = []
                for h in (0, 1):
                    st = sts[h]
                    psB = pspool.tile([P, 2, F], f32, tag="ps2", bufs=3,
                                      name="ps2")
                    psBs.append(psB)
                    for g in (0, 1):
                        s = 2 * sp + g
                        nc.tensor.matmul(psB[:, g:g+1, :], identB[:],
                                         st["x0B"][:, s, :],
                                         start=True, stop=False)
                for h in (0, 1):
                    for g in (0, 1):
                        nc.tensor.matmul(psBs[h][:, g:g+1, :], onesB[:],
                                         nmrowB[:], start=False, stop=False,
                                         skip_group_check=True)
                for kt in range(NF):
                    for h in (0, 1):
                        for g in (0, 1):
                            s = 2 * sp + g
                            nc.tensor.matmul(
                                psBs[h][:, g:g+1, :],
                                sts[h]["h9"][:, kt, s * P:(s + 1) * P],
                                wTb[:, kt * F:(kt + 1) * F],
                                start=False, stop=(kt == NF - 1),
                                skip_group_check=True)
                ss = slice(2 * sp, 2 * sp + 2)
                for h in (0, 1):
                    st = sts[h]
                    outB = wpool.tile([P, 2, F], f32, tag="outB", bufs=2,
                                      name="outB")
                    if evict_beta():
                        eB = wpool.tile([P, 2, F], f32, tag="eB", bufs=2,
                                        name="eB")
                        nc.scalar.copy(eB[:], psBs[h][:])
                        nc.gpsimd.tensor_tensor(outB[:], eB[:],
                                                st["obsB"][:, ss, :], OP.mult)
                    else:
                        nc.vector.tensor_tensor(outB[:], psBs[h][:],
                                                st["obsB"][:, ss, :], OP.mult)
                    for g in (0, 1):
                        s = 2 * sp + g
                        eng = nc.scalar if (last_drain and g == 1) else nc.sync
                        eng.dma_start(
                            out_ap[bts[h] * BT + s * P: bts[h] * BT + (s + 1) * P, :],
                            outB[:, g, :])

            # chunk schedules (same slot machinery as the f32r variant)
            bm_at: dict = {}
            fm_at: dict = {}
            slots = list(range(1, depth - 1))
            if slots:
                for i in range(NF):
                    bm_at.setdefault(slots[(2 * i) % len(slots)], []).append(i)
                    fm_at.setdefault(
                        slots[(2 * i + 1) % len(slots)], []).append(i)
            else:
                bm_at[0] = list(range(NF))
                fm_at[0] = list(range(NF))

            assert nbt % 2 == 0
            npairs = nbt // 2
            xBs = [load(0, 0), load(1, 1)]
            w_loads()
            w_transposes()
            sts = [alloc_st(0, xBs[0]), alloc_st(1, xBs[1])]
            for j in range(NF):
                for h in (0, 1):
                    setup_fm_chunk(sts[h], j, startup=True)
            for pr in range(npairs):
                bts = (2 * pr, 2 * pr + 1)
                nxt = (2 * pr + 2, 2 * pr + 3)
                last = pr + 1 >= npairs
                sts_next = None
                if depth == 1:
                    for h in (0, 1):
                        h9_zero(sts[h])
                        for j in range(NF):
                            nc.vector.copy_predicated(
                                sts[h]["h9"][:, j:j+1, :],
                                sts[h]["obsT"][:, j:j+1, :].bitcast(i16),
                                sts[h]["cth"][:, j:j+1, :])
                        for i in range(NF):
                            setup_bm_chunk(sts[h], i)
                for t in range(depth - 1):
                    round_fm_pair(sts, t)
                    for h in (0, 1):
                        for i in bm_at.get(t, []):
                            setup_bm_chunk(sts[h], i)
                        if sts_next is not None:
                            for j in fm_at.get(t, []):
                                setup_fm_chunk(sts_next[h], j)
                    if t == 0 and not last:
                        xBs = [load(nxt[0], 0), load(nxt[1], 1)]
                        sts_next = [alloc_st(0, xBs[0]), alloc_st(1, xBs[1])]
                        for h in (0, 1):
                            for j in fm_at.get(0, []):
                                setup_fm_chunk(sts_next[h], j)
                # final batch-major round
                for sp in range(NF // 2):
                    bm_group_pair(sts, bts, sp, last_drain=last)
                if depth == 1 and not last:
                    xBs = [load(nxt[0], 0), load(nxt[1], 1)]
                    sts_next = [alloc_st(0, xBs[0]), alloc_st(1, xBs[1])]
                    for h in (0, 1):
                        for j in range(NF):
                            setup_fm_chunk(sts_next[h], j)
                if not last:
                    sts = sts_next

    nc.compile()
    return nc


def _get(rows, depth):
    key = (rows, depth)
    if key not in _cache:
        _cache[key] = _build(rows, depth)
    return _cache[key]


def kernel(x, mu, W, depth):
    from concourse.bass_utils import run_bass_kernel_spmd

    depth = int(depth)
    x = np.ascontiguousarray(x, dtype=np.float32)
    mu = np.ascontiguousarray(mu, dtype=np.float32)
    W = np.ascontiguousarray(W, dtype=np.float32)
    if depth < 1:
        miss = np.isnan(x)
        obs = (~miss).astype(np.float32)
        return np.where(miss, 0.0, x) - obs * mu
    nc = _get(x.shape[0] // N_CORES, depth)
    shards = np.split(x, N_CORES, axis=0)
    in_maps = [{"x": s, "mu": mu, "W": W} for s in shards]
    res = run_bass_kernel_spmd(nc, in_maps, core_ids=list(range(N_CORES)))
    return np.concatenate([res.results[i]["out"] for i in range(N_CORES)],
                          axis=0)
